# revision 24
# baseline (speedup 1.0000x reference)
"""Distributed multi-head attention forward for 8 TRN2 NeuronCores.

Problem: y = proj(softmax((x Wq^T + bq)(x Wk^T + bk)^T / sqrt(hd)) (x Wv^T + bv))
  x: [4, 2048, 1024], 16 heads, head_dim 64, fp32.

Sharding: query-parallel with redundant K/V. Core i owns global flat tokens
[i*1024, (i+1)*1024) as queries (cores 2b, 2b+1 own batch b). Each core
receives the FULL batch activations (its own tokens first, partner's second —
host arranges this so the SPMD graph is core-independent), computes K/V for
all 2048 batch tokens locally (+50% projection FLOPs), Q for its own 1024,
then full 16-head attention for its query slice and the output projection.
No collectives at all; output is token-sharded and concatenated on the host.
(K/V see partner tokens in a consistent order; softmax is permutation-
invariant over keys so the result is exact.)

Layouts (host pre-transposes, all free):
  xT      [D, 2*tq]  feature-major activations (my tokens | partner tokens)
  w_{q,k}T[D, D]     so qT/kT come out feature-major: qT[f, t]
  w_vT    [D, D]     v computed token-major: v[t, f]
  bias    [128, D/128] partition-major (per-partition scalars for ACT bias)
  w_projT [D, D]
  out     yT [D, tq] (host transposes back)

b_v is folded into b_proj on the host: (o + bv) Wp^T + bp = o Wp^T + (Wp bv + bp).

K goes through DRAM as kT [D, 2048] (contiguous per-head-pair rows).
V goes through DRAM head-blocked: vbuf[kt, p, h*65:(h+1)*65] where cols
0:64 of each 65-block are v values for head h, col 64 is a baked-in 1.0
(ones column). The AV matmul then computes both the attention output and
the softmax denominator in one accumulation:
  OT_aug[0:64, q] = sum_k v[k, d] p[k, q]   OT_aug[64, q] = sum_k p[k, q] = Z
Softmax skips the max subtraction (scores ~N(0, 0.17), exp safe in fp32).
Normalization: Z row -> SBUF, reciprocal_approx_fast, gpsimd
partition_broadcast to [64, qch], one fused DVE multiply (PSUM-evict+scale).

All matmul operands are float32r (fp32 bits, PE rounds internally; full
PE rate at free-dim >= 256, ~1.6e-4 rel err vs 4x-slower exact fp32).
"""

import numpy as np

P = 128
D = 1024
NH = 16
HD = 64
SCALE = 1.0 / float(np.sqrt(HD))
NCORES = 8
TQ = 1024          # query tokens per core
B, T = 4, 2048

_COMPILED = {}


def _full_cfg():
    return dict(D=D, NH=NH, TQ=TQ, n_devices=NCORES)


def build(cfg=None):
    """Build + compile the per-core Bass graph. Returns the compiled Bacc."""
    from concourse import bacc
    import concourse.mybir as mybir
    import concourse.tile as tile

    if cfg is None:
        cfg = _full_cfg()
    d = cfg["D"]; nh = cfg["NH"]; tq = cfg["TQ"]
    n_dev = cfg["n_devices"]
    tk = 2 * tq                      # batch tokens for k/v
    f32 = mybir.dt.float32
    bf16 = mybir.dt.bfloat16
    AF = mybir.ActivationFunctionType

    nft = d // P                     # feature tiles (also contraction chunks)
    qch = min(512, tq)               # q free-dim chunk
    nqc = tq // qch
    kch = min(512, tq)               # token chunk for k projection
    nkc = tk // kch
    nkt = tk // P                    # k tiles along batch tokens
    nhp = nh // 2                    # head pairs
    fch = min(512, d)
    nfc = d // fch
    hpf = fch // HD                  # heads per v f-chunk

    nc = bacc.Bacc("TRN2", target_bir_lowering=False, debug=False,
                   num_devices=n_dev)

    xT = nc.dram_tensor("xT", [d, tk], bf16, kind="ExternalInput")
    w_qT = nc.dram_tensor("w_qT", [d, d], bf16, kind="ExternalInput")
    w_kT = nc.dram_tensor("w_kT", [d, d], bf16, kind="ExternalInput")
    w_vT = nc.dram_tensor("w_vT", [d, d], bf16, kind="ExternalInput")
    w_pT = nc.dram_tensor("w_pT", [d, d], bf16, kind="ExternalInput")
    b_q = nc.dram_tensor("b_q", [P, nft], f32, kind="ExternalInput")
    b_k = nc.dram_tensor("b_k", [P, nft], f32, kind="ExternalInput")
    b_p = nc.dram_tensor("b_p", [P, nft], f32, kind="ExternalInput")
    outT = nc.dram_tensor("out", [d, tq], f32, kind="ExternalOutput")

    with tile.TileContext(nc) as tc:
        with (
            tc.tile_pool(name="persist", bufs=1) as persist,
            tc.tile_pool(name="bias", bufs=1) as biasp,
            tc.tile_pool(name="xpool", bufs=2) as xpool,
            tc.tile_pool(name="wpool", bufs=2) as wpool,
            tc.tile_pool(name="ptile", bufs=2) as ptile,
            tc.tile_pool(name="zpool", bufs=1) as zpool,
            tc.tile_pool(name="rzbp", bufs=1) as rzbp,
            tc.tile_pool(name="ypool", bufs=1) as ypool,
            tc.tile_pool(name="psmm", bufs=2, space="PSUM") as psmm,
            tc.tile_pool(name="pst", bufs=2, space="PSUM") as pst,
            tc.tile_pool(name="pot", bufs=2, space="PSUM") as pot,
        ):
            # ---- persistent SBUF ----
            q_all = persist.tile([P, nft, tq], bf16)     # qT, feature-major
            kt_all = persist.tile([P, nhp, tk], bf16)    # kT by head pair
            vt_all = persist.tile([P, nkt, nh * (HD + 1)], bf16)  # v + ones col
            ot_all = persist.tile([P, nft, tq], bf16)    # attention out^T
            wp_sb = persist.tile([P, nft, d], bf16)
            bq_sb = biasp.tile([P, nft], f32)
            nc.sync.dma_start(bq_sb[:], b_q[:])
            bk_sb = biasp.tile([P, nft], f32)
            nc.sync.dma_start(bk_sb[:], b_k[:])
            bp_sb = biasp.tile([P, nft], f32)
            nc.sync.dma_start(bp_sb[:], b_p[:])

            # x for both batch halves, feature-major, chunked by d so the
            # first matmuls can start before the whole input has landed
            xh = []
            for half in range(tk // tq):
                x_sb = xpool.tile([P, nft, tq], bf16, tag="x")
                for dc in range(nft):
                    nc.sync.dma_start(
                        x_sb[:, dc, :],
                        xT[dc * P:(dc + 1) * P, half * tq:(half + 1) * tq])
                xh.append(x_sb)

            # ones columns for the AV sum-of-exp trick
            nc.vector.memset(vt_all[:].rearrange("p k (h e) -> p k h e",
                                                 e=HD + 1)[:, :, :, HD], 1.0)

            # ---- projection work units (k/q/v/out-proj), emitted
            # interleaved with attention pairs so projection matmuls fill
            # the PE gaps in the ACT(exp)-bound attention stream
            wk_sb = wpool.tile([P, nft, d], bf16, tag="w", bufs=3)
            for dc in range(nft):
                nc.sync.dma_start(wk_sb[:, dc, :], w_kT[dc * P:(dc + 1) * P, :])
            wv_sb = wpool.tile([P, nft, d], bf16, tag="w", bufs=3)
            for dc in range(nft):
                nc.sync.dma_start(wv_sb[:, dc, :], w_vT[dc * P:(dc + 1) * P, :])
            wq_sb = wpool.tile([P, nft, d], bf16, tag="w", bufs=3)
            for dc in range(nft):
                nc.sync.dma_start(wq_sb[:, dc, :], w_qT[dc * P:(dc + 1) * P, :])
            for dc in range(nft):
                nc.sync.dma_start(wp_sb[:, dc, :], w_pT[dc * P:(dc + 1) * P, :])

            def k_unit(ft, c):
                xsb = xh[c * kch // tq]
                t0 = (c * kch) % tq
                ps = psmm.tile([P, kch], f32, tag="mm", name="ps_k")
                for dc in range(nft):
                    nc.tensor.matmul(
                        ps[:],
                        wk_sb[:, dc, ft * P:(ft + 1) * P],
                        xsb[:, dc, t0:t0 + kch],
                        start=(dc == 0), stop=(dc == nft - 1))
                nc.vector.tensor_scalar_add(
                    kt_all[:, ft, c * kch:(c + 1) * kch], ps[:],
                    bk_sb[:, ft:ft + 1])

            def q_unit(ft, c):
                ps = psmm.tile([P, qch], f32, tag="mm", name="ps_q")
                for dc in range(nft):
                    nc.tensor.matmul(
                        ps[:],
                        wq_sb[:, dc, ft * P:(ft + 1) * P],
                        xh[0][:, dc, c * qch:(c + 1) * qch],
                        start=(dc == 0), stop=(dc == nft - 1))
                nc.vector.tensor_scalar_add(
                    q_all[:, ft, c * qch:(c + 1) * qch], ps[:],
                    bq_sb[:, ft:ft + 1])

            def v_unit(fc, tt):
                xsb = xh[tt * P // tq]
                tcol = (tt * P) % tq
                ps = psmm.tile([P, fch], f32, tag="mm", name="ps_v")
                for dc in range(nft):
                    nc.tensor.matmul(
                        ps[:],
                        xsb[:, dc, tcol:tcol + P],
                        wv_sb[:, dc, fc * fch:(fc + 1) * fch],
                        start=(dc == 0), stop=(dc == nft - 1))
                dst = (vt_all[:, tt, fc * hpf * (HD + 1):(fc + 1) * hpf * (HD + 1)]
                       .rearrange("p (h e) -> p h e", e=HD + 1)[:, :, 0:HD])
                nc.vector.tensor_copy(
                    dst, ps[:].rearrange("p (h e) -> p h e", e=HD))

            def proj_unit(qc, jt):
                ps = psmm.tile([P, qch], f32, tag="mm", name="ps_p")
                for dc in range(nft):
                    nc.tensor.matmul(
                        ps[:],
                        wp_sb[:, dc, jt * P:(jt + 1) * P],
                        ot_all[:, dc, qc * qch:(qc + 1) * qch],
                        start=(dc == 0), stop=(dc == nft - 1))
                ysb = ypool.tile([P, qch], f32, name="ysb")
                nc.vector.tensor_scalar_add(ysb[:], ps[:], bp_sb[:, jt:jt + 1])
                nc.sync.dma_start(
                    outT[jt * P:(jt + 1) * P, qc * qch:(qc + 1) * qch],
                    ysb[:])

            def attn_pair(hp, qc):
                hA, hB = 2 * hp, 2 * hp + 1
                otA = pot.tile([P, qch], f32, tag="ot", name="otA")
                otB = pot.tile([P, qch], f32, tag="ot", name="otB")
                qA = q_all[0:HD, hp, qc * qch:(qc + 1) * qch]
                qB = q_all[HD:2 * HD, hp, qc * qch:(qc + 1) * qch]

                def emit_av(k, pt):
                    for (ot, h, p0) in ((otA, hA, 0), (otB, hB, qch)):
                        nc.tensor.matmul(
                            ot[0:HD + 1, :],
                            vt_all[:, k, h * (HD + 1):(h + 1) * (HD + 1)],
                            pt[:, p0:p0 + qch],
                            start=(k == 0), stop=(k == nkt - 1))

                pend = None
                for k in range(nkt):
                    st = pst.tile([P, 2 * qch], f32, tag="st", name="st")
                    nc.tensor.matmul(
                        st[:, 0:qch],
                        kt_all[0:HD, hp, k * P:(k + 1) * P],
                        qA, start=True, stop=True)
                    nc.tensor.matmul(
                        st[:, qch:2 * qch],
                        kt_all[HD:2 * HD, hp, k * P:(k + 1) * P],
                        qB, start=True, stop=True)
                    pt = ptile.tile([P, 2 * qch], bf16, tag="pt", name="pt")
                    nc.scalar.activation(pt[:], st[:], AF.Exp, scale=SCALE)
                    if pend is not None:
                        emit_av(*pend)
                    pend = (k, pt)
                emit_av(*pend)

                for (ot, hh) in ((otA, 0), (otB, 1)):
                    zrow = zpool.tile([1, qch], f32, tag="z", name="zrow")
                    nc.vector.tensor_copy(zrow[:], ot[HD:HD + 1, :])
                    rz = zpool.tile([1, qch], f32, tag="z2", name="rz")
                    nc.vector.reciprocal_approx_fast(rz[:], zrow[:])
                    rzb = rzbp.tile([HD, qch], f32, name="rzb")
                    nc.gpsimd.partition_broadcast(rzb[:], rz[:])
                    nc.vector.tensor_mul(
                        ot_all[hh * HD:(hh + 1) * HD, hp,
                               qc * qch:(qc + 1) * qch],
                        ot[0:HD, :], rzb[:])

            # ---- the interleaved schedule ----
            # kq(ft) and v(fc) units ahead of the attention pairs that need
            # them; later projection units slot between attention pairs
            def kq(ft):
                for c in range(nkc):
                    k_unit(ft, c)
                for c in range(nqc):
                    q_unit(ft, c)

            kq(0)
            for tt in range(nkt):
                v_unit(0, tt)
            attn_pair(0, 0)
            if nqc > 1:
                attn_pair(0, 1)
            if nhp > 1:
                kq(1)

            # filler queues: remaining v chunks and k/q feature tiles.
            # Required units are force-drained before the pair that reads
            # them; otherwise one filler is emitted after each pair to keep
            # PE fed during the ACT-bound attention stream.
            pending_v = [(fc, tt) for fc in range(1, nfc) for tt in range(nkt)]
            pending_kq = list(range(2, nhp))

            def emit_required(hp):
                need_fc = (2 * hp) // hpf
                while pending_kq and pending_kq[0] <= hp:
                    kq(pending_kq.pop(0))
                while pending_v and pending_v[0][0] <= need_fc:
                    fc, tt = pending_v.pop(0)
                    v_unit(fc, tt)

            def emit_filler(n):
                for _ in range(n):
                    if pending_v:
                        fc, tt = pending_v.pop(0)
                        v_unit(fc, tt)
                    elif pending_kq:
                        kq(pending_kq.pop(0))

            # qc0 pairs first (fed by remaining projection units), then the
            # qc0 output projection doubles as PE filler for the qc1 pairs
            pairs0 = [(hp, 0) for hp in range(1, nhp)]
            nfill = len(pending_v) + 2 * len(pending_kq)
            per = max(1, (nfill + len(pairs0) - 1) // max(1, len(pairs0)))
            for (hp, qc) in pairs0:
                emit_required(hp)
                attn_pair(hp, qc)
                emit_filler(per)
            emit_filler(len(pending_v) + len(pending_kq))
            if nqc > 1:
                proj_q = list(range(nft))
                for hp in range(nhp):
                    attn_pair(hp, 1)
                    if hp > 0:
                        for _ in range(nft // (nhp - 1) + 1):
                            if proj_q:
                                proj_unit(0, proj_q.pop(0))
                while proj_q:
                    proj_unit(0, proj_q.pop(0))
                for jt in range(nft):
                    proj_unit(1, jt)
            else:
                for jt in range(nft):
                    proj_unit(0, jt)

    nc.compile()
    return nc


def make_in_maps(inputs, cfg=None):
    """Host-side sharding: full inputs -> per-core input dicts."""
    if cfg is None:
        cfg = _full_cfg()
    d = cfg["D"]; tq = cfg["TQ"]; n_dev = cfg["n_devices"]; nh = cfg["NH"]
    nft = d // P
    nkt = 2 * tq // P

    x = np.asarray(inputs["x"], dtype=np.float32)
    w_qkv = np.asarray(inputs["w_qkv"], dtype=np.float32)
    b_qkv = np.asarray(inputs["b_qkv"], dtype=np.float32)
    w_proj = np.asarray(inputs["w_proj"], dtype=np.float32)
    b_proj = np.asarray(inputs["b_proj"], dtype=np.float32)

    import ml_dtypes
    bf = ml_dtypes.bfloat16

    x_flat = x.reshape(-1, d)
    w_qT = np.ascontiguousarray(w_qkv[0:d].T).astype(bf)
    w_kT = np.ascontiguousarray(w_qkv[d:2 * d].T).astype(bf)
    w_vT = np.ascontiguousarray(w_qkv[2 * d:3 * d].T).astype(bf)
    b_q = b_qkv[0:d]; b_k = b_qkv[d:2 * d]; b_v = b_qkv[2 * d:3 * d]
    w_pT = np.ascontiguousarray(w_proj.T).astype(bf)
    b_p_eff = b_proj + w_proj @ b_v

    def bias_tile(b):
        return np.ascontiguousarray(b.reshape(nft, P).T)

    shared = {
        "w_qT": w_qT, "w_kT": w_kT, "w_vT": w_vT, "w_pT": w_pT,
        "b_q": bias_tile(b_q), "b_k": bias_tile(b_k), "b_p": bias_tile(b_p_eff),
    }
    in_maps = []
    for i in range(n_dev):
        mine = x_flat[i * tq:(i + 1) * tq]
        partner = x_flat[(i ^ 1) * tq:((i ^ 1) + 1) * tq]
        xT_i = np.ascontiguousarray(
            np.concatenate([mine, partner], axis=0).T).astype(bf)
        in_maps.append({"xT": xT_i, **shared})
    return in_maps


def assemble_output(results, inputs, cfg=None):
    if cfg is None:
        cfg = _full_cfg()
    d = cfg["D"]; tq = cfg["TQ"]; n_dev = cfg["n_devices"]
    x = np.asarray(inputs["x"])
    y = np.empty((n_dev * tq, d), dtype=np.float32)
    for i in range(n_dev):
        y[i * tq:(i + 1) * tq] = results[i]["out"].T
    return y.reshape(x.shape)


def run(inputs, trace=False, **kw):
    from concourse.bass_utils import run_bass_kernel_spmd
    key = "full"
    if key not in _COMPILED:
        _COMPILED[key] = build()
    nc = _COMPILED[key]
    in_maps = make_in_maps(inputs)
    res = run_bass_kernel_spmd(nc, in_maps, core_ids=list(range(NCORES)),
                               trace=trace, **kw)
    return res


def kernel(**inputs) -> np.ndarray:
    res = run(inputs, trace=False)
    return assemble_output(res.results, inputs)


# revision 27
# speedup vs baseline: 1.1142x; 1.1142x over previous
"""Distributed multi-head attention forward for 8 TRN2 NeuronCores.

Problem: y = proj(softmax((x Wq^T + bq)(x Wk^T + bk)^T / sqrt(hd)) (x Wv^T + bv))
  x: [4, 2048, 1024], 16 heads, head_dim 64, fp32.

Sharding: query-parallel with redundant K/V. Core i owns global flat tokens
[i*1024, (i+1)*1024) as queries (cores 2b, 2b+1 own batch b). Each core
receives the FULL batch activations (its own tokens first, partner's second —
host arranges this so the SPMD graph is core-independent), computes K/V for
all 2048 batch tokens locally (+50% projection FLOPs), Q for its own 1024,
then full 16-head attention for its query slice and the output projection.
No collectives at all; output is token-sharded and concatenated on the host.
(K/V see partner tokens in a consistent order; softmax is permutation-
invariant over keys so the result is exact.)

Layouts (host pre-transposes, all free):
  xT      [D, 2*tq]  feature-major activations (my tokens | partner tokens)
  w_{q,k}T[D, D]     so qT/kT come out feature-major: qT[f, t]
  w_vT    [D, D]     v computed token-major: v[t, f]
  bias    [128, D/128] partition-major (per-partition scalars for ACT bias)
  w_projT [D, D]
  out     yT [D, tq] (host transposes back)

b_v is folded into b_proj on the host: (o + bv) Wp^T + bp = o Wp^T + (Wp bv + bp).

K goes through DRAM as kT [D, 2048] (contiguous per-head-pair rows).
V goes through DRAM head-blocked: vbuf[kt, p, h*65:(h+1)*65] where cols
0:64 of each 65-block are v values for head h, col 64 is a baked-in 1.0
(ones column). The AV matmul then computes both the attention output and
the softmax denominator in one accumulation:
  OT_aug[0:64, q] = sum_k v[k, d] p[k, q]   OT_aug[64, q] = sum_k p[k, q] = Z
Softmax skips the max subtraction (scores ~N(0, 0.17), exp safe in fp32).
Normalization: Z row -> SBUF, reciprocal_approx_fast, gpsimd
partition_broadcast to [64, qch], one fused DVE multiply (PSUM-evict+scale).

All matmul operands are float32r (fp32 bits, PE rounds internally; full
PE rate at free-dim >= 256, ~1.6e-4 rel err vs 4x-slower exact fp32).
"""

import numpy as np

P = 128
D = 1024
NH = 16
HD = 64
SCALE = 1.0 / float(np.sqrt(HD))
NCORES = 8
TQ = 1024          # query tokens per core
B, T = 4, 2048

_COMPILED = {}


def _full_cfg():
    return dict(D=D, NH=NH, TQ=TQ, n_devices=NCORES)


def build(cfg=None):
    """Build + compile the per-core Bass graph. Returns the compiled Bacc."""
    from concourse import bacc
    import concourse.mybir as mybir
    import concourse.tile as tile

    if cfg is None:
        cfg = _full_cfg()
    d = cfg["D"]; nh = cfg["NH"]; tq = cfg["TQ"]
    n_dev = cfg["n_devices"]
    tk = 2 * tq                      # batch tokens for k/v
    f32 = mybir.dt.float32
    bf16 = mybir.dt.bfloat16
    AF = mybir.ActivationFunctionType

    nft = d // P                     # feature tiles (also contraction chunks)
    qch = min(512, tq)               # q free-dim chunk
    nqc = tq // qch
    kch = min(512, tq)               # token chunk for k projection
    nkc = tk // kch
    nkt = tk // P                    # k tiles along batch tokens
    nhp = nh // 2                    # head pairs
    fch = min(512, d)
    nfc = d // fch
    hpf = fch // HD                  # heads per v f-chunk

    nc = bacc.Bacc("TRN2", target_bir_lowering=False, debug=False,
                   num_devices=n_dev)

    xT = nc.dram_tensor("xT", [d, tk], bf16, kind="ExternalInput")
    w_qT = nc.dram_tensor("w_qT", [d, d], bf16, kind="ExternalInput")
    w_kT = nc.dram_tensor("w_kT", [d, d], bf16, kind="ExternalInput")
    w_vT = nc.dram_tensor("w_vT", [d, d], bf16, kind="ExternalInput")
    w_pT = nc.dram_tensor("w_pT", [d, d], bf16, kind="ExternalInput")
    b_q = nc.dram_tensor("b_q", [P, nft], f32, kind="ExternalInput")
    b_k = nc.dram_tensor("b_k", [P, nft], f32, kind="ExternalInput")
    b_p = nc.dram_tensor("b_p", [P, nft], f32, kind="ExternalInput")
    outT = nc.dram_tensor("out", [d, tq], f32, kind="ExternalOutput")

    with tile.TileContext(nc) as tc:
        with (
            tc.tile_pool(name="persist", bufs=1) as persist,
            tc.tile_pool(name="bias", bufs=1) as biasp,
            tc.tile_pool(name="xpool", bufs=2) as xpool,
            tc.tile_pool(name="wpool", bufs=2) as wpool,
            tc.tile_pool(name="ptile", bufs=3) as ptile,
            tc.tile_pool(name="zpool", bufs=1) as zpool,
            tc.tile_pool(name="rzbp", bufs=1) as rzbp,
            tc.tile_pool(name="ypool", bufs=1) as ypool,
            tc.tile_pool(name="psmm", bufs=2, space="PSUM") as psmm,
            tc.tile_pool(name="pst", bufs=2, space="PSUM") as pst,
            tc.tile_pool(name="pot", bufs=2, space="PSUM") as pot,
        ):
            # ---- persistent SBUF ----
            q_all = persist.tile([P, nft, tq], bf16)     # qT, feature-major
            kt_all = persist.tile([P, nhp, tk], bf16)    # kT by head pair
            vt_all = persist.tile([P, nkt, nh * (HD + 1)], bf16)  # v + ones col
            ot_all = persist.tile([P, nft, tq], bf16)    # attention out^T
            wp_sb = persist.tile([P, nft, d], bf16)
            bq_sb = biasp.tile([P, nft], f32)
            nc.sync.dma_start(bq_sb[:], b_q[:])
            bk_sb = biasp.tile([P, nft], f32)
            nc.sync.dma_start(bk_sb[:], b_k[:])
            bp_sb = biasp.tile([P, nft], f32)
            nc.sync.dma_start(bp_sb[:], b_p[:])

            # x for both batch halves, feature-major, chunked by d so the
            # first matmuls can start before the whole input has landed
            xh = []
            for half in range(tk // tq):
                x_sb = xpool.tile([P, nft, tq], bf16, tag="x")
                for dc in range(nft):
                    nc.sync.dma_start(
                        x_sb[:, dc, :],
                        xT[dc * P:(dc + 1) * P, half * tq:(half + 1) * tq])
                xh.append(x_sb)

            # ones columns for the AV sum-of-exp trick
            nc.vector.memset(vt_all[:].rearrange("p k (h e) -> p k h e",
                                                 e=HD + 1)[:, :, :, HD], 1.0)

            # ---- projection work units (k/q/v/out-proj), emitted
            # interleaved with attention pairs so projection matmuls fill
            # the PE gaps in the ACT(exp)-bound attention stream
            wk_sb = wpool.tile([P, nft, d], bf16, tag="w", bufs=3)
            for dc in range(nft):
                nc.sync.dma_start(wk_sb[:, dc, :], w_kT[dc * P:(dc + 1) * P, :])
            wv_sb = wpool.tile([P, nft, d], bf16, tag="w", bufs=3)
            for dc in range(nft):
                nc.sync.dma_start(wv_sb[:, dc, :], w_vT[dc * P:(dc + 1) * P, :])
            wq_sb = wpool.tile([P, nft, d], bf16, tag="w", bufs=3)
            for dc in range(nft):
                nc.sync.dma_start(wq_sb[:, dc, :], w_qT[dc * P:(dc + 1) * P, :])
            for dc in range(nft):
                nc.sync.dma_start(wp_sb[:, dc, :], w_pT[dc * P:(dc + 1) * P, :])

            def k_unit(ft, c):
                xsb = xh[c * kch // tq]
                t0 = (c * kch) % tq
                ps = psmm.tile([P, kch], f32, tag="mm", name="ps_k")
                for dc in range(nft):
                    nc.tensor.matmul(
                        ps[:],
                        wk_sb[:, dc, ft * P:(ft + 1) * P],
                        xsb[:, dc, t0:t0 + kch],
                        start=(dc == 0), stop=(dc == nft - 1))
                nc.vector.tensor_scalar_add(
                    kt_all[:, ft, c * kch:(c + 1) * kch], ps[:],
                    bk_sb[:, ft:ft + 1])

            def q_unit(ft, c):
                ps = psmm.tile([P, qch], f32, tag="mm", name="ps_q")
                for dc in range(nft):
                    nc.tensor.matmul(
                        ps[:],
                        wq_sb[:, dc, ft * P:(ft + 1) * P],
                        xh[0][:, dc, c * qch:(c + 1) * qch],
                        start=(dc == 0), stop=(dc == nft - 1))
                nc.vector.tensor_scalar_add(
                    q_all[:, ft, c * qch:(c + 1) * qch], ps[:],
                    bq_sb[:, ft:ft + 1])

            def v_unit(fc, tt):
                xsb = xh[tt * P // tq]
                tcol = (tt * P) % tq
                ps = psmm.tile([P, fch], f32, tag="mm", name="ps_v")
                for dc in range(nft):
                    nc.tensor.matmul(
                        ps[:],
                        xsb[:, dc, tcol:tcol + P],
                        wv_sb[:, dc, fc * fch:(fc + 1) * fch],
                        start=(dc == 0), stop=(dc == nft - 1))
                dst = (vt_all[:, tt, fc * hpf * (HD + 1):(fc + 1) * hpf * (HD + 1)]
                       .rearrange("p (h e) -> p h e", e=HD + 1)[:, :, 0:HD])
                nc.vector.tensor_copy(
                    dst, ps[:].rearrange("p (h e) -> p h e", e=HD))

            def proj_unit(qc, jt):
                ps = psmm.tile([P, qch], f32, tag="mm", name="ps_p")
                for dc in range(nft):
                    nc.tensor.matmul(
                        ps[:],
                        wp_sb[:, dc, jt * P:(jt + 1) * P],
                        ot_all[:, dc, qc * qch:(qc + 1) * qch],
                        start=(dc == 0), stop=(dc == nft - 1))
                ysb = ypool.tile([P, qch], f32, name="ysb")
                nc.vector.tensor_scalar_add(ysb[:], ps[:], bp_sb[:, jt:jt + 1])
                nc.sync.dma_start(
                    outT[jt * P:(jt + 1) * P, qc * qch:(qc + 1) * qch],
                    ysb[:])

            def attn_pair(hp, qc):
                hA, hB = 2 * hp, 2 * hp + 1
                otA = pot.tile([P, qch], f32, tag="ot", name="otA")
                otB = pot.tile([P, qch], f32, tag="ot", name="otB")
                qA = q_all[0:HD, hp, qc * qch:(qc + 1) * qch]
                qB = q_all[HD:2 * HD, hp, qc * qch:(qc + 1) * qch]

                def emit_av(k, pt):
                    for (ot, h, p0) in ((otA, hA, 0), (otB, hB, qch)):
                        nc.tensor.matmul(
                            ot[0:HD + 1, :],
                            vt_all[:, k, h * (HD + 1):(h + 1) * (HD + 1)],
                            pt[:, p0:p0 + qch],
                            start=(k == 0), stop=(k == nkt - 1))

                pend = None
                for k in range(nkt):
                    st = pst.tile([P, 2 * qch], f32, tag="st", name="st")
                    nc.tensor.matmul(
                        st[:, 0:qch],
                        kt_all[0:HD, hp, k * P:(k + 1) * P],
                        qA, start=True, stop=True)
                    nc.tensor.matmul(
                        st[:, qch:2 * qch],
                        kt_all[HD:2 * HD, hp, k * P:(k + 1) * P],
                        qB, start=True, stop=True)
                    pt = ptile.tile([P, 2 * qch], bf16, tag="pt", name="pt")
                    nc.scalar.activation(pt[:], st[:], AF.Exp, scale=SCALE)
                    if pend is not None:
                        emit_av(*pend)
                    pend = (k, pt)
                emit_av(*pend)

                for (ot, hh) in ((otA, 0), (otB, 1)):
                    zrow = zpool.tile([1, qch], f32, tag="z", name="zrow")
                    nc.vector.tensor_copy(zrow[:], ot[HD:HD + 1, :])
                    rz = zpool.tile([1, qch], f32, tag="z2", name="rz")
                    nc.vector.reciprocal_approx_fast(rz[:], zrow[:])
                    rzb = rzbp.tile([HD, qch], f32, name="rzb")
                    nc.gpsimd.partition_broadcast(rzb[:], rz[:])
                    nc.vector.tensor_mul(
                        ot_all[hh * HD:(hh + 1) * HD, hp,
                               qc * qch:(qc + 1) * qch],
                        ot[0:HD, :], rzb[:])

            # ---- the interleaved schedule ----
            # kq(ft) and v(fc) units ahead of the attention pairs that need
            # them; later projection units slot between attention pairs
            def kq(ft):
                for c in range(nkc):
                    k_unit(ft, c)
                for c in range(nqc):
                    q_unit(ft, c)

            kq(0)
            for tt in range(nkt):
                v_unit(0, tt)
            attn_pair(0, 0)
            if nqc > 1:
                attn_pair(0, 1)
            if nhp > 1:
                kq(1)

            # filler queues: remaining v chunks and k/q feature tiles.
            # Required units are force-drained before the pair that reads
            # them; otherwise one filler is emitted after each pair to keep
            # PE fed during the ACT-bound attention stream.
            pending_v = [(fc, tt) for fc in range(1, nfc) for tt in range(nkt)]
            pending_kq = list(range(2, nhp))

            def emit_required(hp):
                need_fc = (2 * hp) // hpf
                while pending_kq and pending_kq[0] <= hp:
                    kq(pending_kq.pop(0))
                while pending_v and pending_v[0][0] <= need_fc:
                    fc, tt = pending_v.pop(0)
                    v_unit(fc, tt)

            def emit_filler(n):
                for _ in range(n):
                    if pending_v:
                        fc, tt = pending_v.pop(0)
                        v_unit(fc, tt)
                    elif pending_kq:
                        kq(pending_kq.pop(0))

            pairs = [(hp, qc) for hp in range(1, nhp) for qc in range(nqc)]
            nfill = len(pending_v) + 2 * len(pending_kq)
            per = max(1, (nfill + len(pairs) - 1) // max(1, len(pairs)))
            for (hp, qc) in pairs:
                emit_required(hp)
                attn_pair(hp, qc)
                emit_filler(per)
                if nqc > 1 and hp == nhp - 1 and qc == 0:
                    emit_filler(len(pending_v) + len(pending_kq))
                    for jt in range(nft):
                        proj_unit(0, jt)
            emit_filler(len(pending_v) + len(pending_kq))
            if nqc > 1:
                for jt in range(nft):
                    proj_unit(1, jt)
            else:
                for jt in range(nft):
                    proj_unit(0, jt)

    nc.compile()
    return nc


def make_in_maps(inputs, cfg=None):
    """Host-side sharding: full inputs -> per-core input dicts."""
    if cfg is None:
        cfg = _full_cfg()
    d = cfg["D"]; tq = cfg["TQ"]; n_dev = cfg["n_devices"]; nh = cfg["NH"]
    nft = d // P
    nkt = 2 * tq // P

    x = np.asarray(inputs["x"], dtype=np.float32)
    w_qkv = np.asarray(inputs["w_qkv"], dtype=np.float32)
    b_qkv = np.asarray(inputs["b_qkv"], dtype=np.float32)
    w_proj = np.asarray(inputs["w_proj"], dtype=np.float32)
    b_proj = np.asarray(inputs["b_proj"], dtype=np.float32)

    import ml_dtypes
    bf = ml_dtypes.bfloat16

    x_flat = x.reshape(-1, d)
    w_qT = np.ascontiguousarray(w_qkv[0:d].T).astype(bf)
    w_kT = np.ascontiguousarray(w_qkv[d:2 * d].T).astype(bf)
    w_vT = np.ascontiguousarray(w_qkv[2 * d:3 * d].T).astype(bf)
    b_q = b_qkv[0:d]; b_k = b_qkv[d:2 * d]; b_v = b_qkv[2 * d:3 * d]
    w_pT = np.ascontiguousarray(w_proj.T).astype(bf)
    b_p_eff = b_proj + w_proj @ b_v

    def bias_tile(b):
        return np.ascontiguousarray(b.reshape(nft, P).T)

    shared = {
        "w_qT": w_qT, "w_kT": w_kT, "w_vT": w_vT, "w_pT": w_pT,
        "b_q": bias_tile(b_q), "b_k": bias_tile(b_k), "b_p": bias_tile(b_p_eff),
    }
    in_maps = []
    for i in range(n_dev):
        mine = x_flat[i * tq:(i + 1) * tq]
        partner = x_flat[(i ^ 1) * tq:((i ^ 1) + 1) * tq]
        xT_i = np.ascontiguousarray(
            np.concatenate([mine, partner], axis=0).T).astype(bf)
        in_maps.append({"xT": xT_i, **shared})
    return in_maps


def assemble_output(results, inputs, cfg=None):
    if cfg is None:
        cfg = _full_cfg()
    d = cfg["D"]; tq = cfg["TQ"]; n_dev = cfg["n_devices"]
    x = np.asarray(inputs["x"])
    y = np.empty((n_dev * tq, d), dtype=np.float32)
    for i in range(n_dev):
        y[i * tq:(i + 1) * tq] = results[i]["out"].T
    return y.reshape(x.shape)


def run(inputs, trace=False, **kw):
    from concourse.bass_utils import run_bass_kernel_spmd
    key = "full"
    if key not in _COMPILED:
        _COMPILED[key] = build()
    nc = _COMPILED[key]
    in_maps = make_in_maps(inputs)
    res = run_bass_kernel_spmd(nc, in_maps, core_ids=list(range(NCORES)),
                               trace=trace, **kw)
    return res


def kernel(**inputs) -> np.ndarray:
    res = run(inputs, trace=False)
    return assemble_output(res.results, inputs)
